# revision 15
# baseline (speedup 1.0000x reference)
"""Trainium2 Bass kernel for nn_BottleneckBlock (quaternion bottleneck block).

Strategy: data-parallel over batch (B=8 -> 8 NeuronCores, 1 image each).
Per core, three phases in ONE NEFF:
  A: stream x, per-(channel,component)-row mean/E[x^2] via bn_stats/bn_aggr,
     AllReduce tiny stats across cores, fold gamma/beta -> per-row affine.
  B: stream x again, fused BN1-affine+SiLU on ScalarE, 1x1 quaternion conv as
     matmuls (Hamilton block matrix precomputed on host), write out1 to DRAM
     while accumulating BN2 stats; AllReduce, fold -> affine2.
  C: sliding row-window over out1 with zero-padded columns, fused
     BN2-affine+SiLU, 3x3 quaternion conv as 9 shifted matmuls accumulating
     in PSUM, write out2.
Host assembles concat([x, out2]) (pure data movement).
"""

import numpy as np

import concourse.bacc as bacc
import concourse.tile as tile
from concourse import mybir
from concourse.bass_utils import run_bass_kernel_spmd

F32 = mybir.dt.float32
F32R = mybir.dt.float32r
AF = mybir.ActivationFunctionType
EPS = 1e-5

N_CORES = 8
C1 = 64          # input quaternion channels
Q = 4
INTER = 128      # intermediate quaternion channels (out_planes*4)
O2 = 32          # output quaternion channels
R1 = C1 * Q      # 256 rows of x
R2 = INTER * Q   # 512 rows of out1
M2 = O2 * Q      # 128 rows of out2
H = W = 128


def enable_ldw_opt():
    """Rewrite walrus's --enable-ldw-opt=false to true (dedupes repeated
    identical LDWEIGHTS; our matmul order repeats weights back-to-back)."""
    import concourse.bass_utils as _bu

    if getattr(_bu, "_ldw_patched", False):
        return
    _orig = _bu.run_command

    def _patched(argv, **kw):
        argv = [
            "--enable-ldw-opt=true" if a == "--enable-ldw-opt=false" else a
            for a in argv
        ]
        return _orig(argv, **kw)

    _bu.run_command = _patched
    _bu._ldw_patched = True


def _affine_from_stats(nc, pool, statg, g_sb, b_sb, nb, eps_t):
    """statg: [128, nb, 2] group-averaged (mean, E[x^2]) per row.
    Returns (scale, shift) [128, nb] tiles with scale=gamma*rsqrt(var+eps),
    shift=beta-mean*scale. rsqrt = ACT sqrt + DVE reciprocal + 2 Newton steps
    (ACT sqrt alone has a loose precision budget)."""
    mean = statg[:, :, 0]
    e2 = statg[:, :, 1]
    vpe = pool.tile([128, nb], F32, tag=f"vpe{nb}")
    tmp = pool.tile([128, nb], F32, tag=f"ntmp{nb}")
    r = pool.tile([128, nb], F32, tag=f"nr{nb}")
    scale = pool.tile([128, nb], F32, tag=f"scale{nb}")
    shift = pool.tile([128, nb], F32, tag=f"shift{nb}")
    # vpe = E2 - mean^2 + eps
    nc.vector.tensor_tensor(out=tmp, in0=mean, in1=mean, op=mybir.AluOpType.mult)
    nc.vector.tensor_tensor(out=vpe, in0=e2, in1=tmp, op=mybir.AluOpType.subtract)
    nc.scalar.activation(out=r, in_=vpe, func=AF.Sqrt, bias=eps_t)
    nc.vector.tensor_scalar_add(out=vpe, in0=vpe, scalar1=float(EPS))
    nc.vector.reciprocal(out=r, in_=r)
    for _ in range(2):
        # r <- r * (1.5 - 0.5 * vpe * r^2)
        nc.vector.tensor_tensor(out=tmp, in0=r, in1=r, op=mybir.AluOpType.mult)
        nc.vector.tensor_tensor(out=tmp, in0=tmp, in1=vpe, op=mybir.AluOpType.mult)
        nc.vector.tensor_scalar(
            out=tmp, in0=tmp, scalar1=-0.5, scalar2=1.5,
            op0=mybir.AluOpType.mult, op1=mybir.AluOpType.add,
        )
        nc.vector.tensor_tensor(out=r, in0=r, in1=tmp, op=mybir.AluOpType.mult)
    nc.vector.tensor_tensor(out=scale, in0=g_sb, in1=r, op=mybir.AluOpType.mult)
    nc.vector.tensor_tensor(out=shift, in0=mean, in1=scale, op=mybir.AluOpType.mult)
    nc.vector.tensor_tensor(out=shift, in0=b_sb, in1=shift, op=mybir.AluOpType.subtract)
    return scale, shift


def build_nc(n_cores=N_CORES, h=H, w=W, use_silu=True, use_f32r=False):
    px = h * w
    assert px % 512 == 0
    mmdt = F32R if use_f32r else F32
    nc = bacc.Bacc("TRN2", target_bir_lowering=False, debug=False, num_devices=n_cores)

    x_ap = nc.dram_tensor("x", [R1, px], F32, kind="ExternalInput").ap()
    w1t_ap = nc.dram_tensor("w1t", [128, 2, R2], F32, kind="ExternalInput").ap()
    w2t_ap = nc.dram_tensor("w2t", [128, 4, 9, M2], F32, kind="ExternalInput").ap()
    gmat_ap = nc.dram_tensor("gmat", [128, 128], F32, kind="ExternalInput").ap()
    g1_ap = nc.dram_tensor("g1", [128, 2], F32, kind="ExternalInput").ap()
    b1_ap = nc.dram_tensor("b1", [128, 2], F32, kind="ExternalInput").ap()
    g2_ap = nc.dram_tensor("g2", [128, 4], F32, kind="ExternalInput").ap()
    b2_ap = nc.dram_tensor("b2", [128, 4], F32, kind="ExternalInput").ap()
    out2_ap = nc.dram_tensor("out2", [M2, px], F32, kind="ExternalOutput").ap()

    groups = [list(range(n_cores))]

    with tile.TileContext(nc) as tc:
        with (
            tc.tile_pool(name="singles", bufs=1) as singles,
            tc.tile_pool(name="pA", bufs=3) as pA,
            tc.tile_pool(name="pB", bufs=3) as pB,
            tc.tile_pool(name="pB1", bufs=6) as pB1,
            tc.tile_pool(name="pC", bufs=2) as pC,
            tc.tile_pool(name="pC2", bufs=4) as pC2,
            tc.tile_pool(name="psum", bufs=6, space="PSUM") as psum,
            tc.tile_pool(name="dram", bufs=1, space="DRAM") as dramp,
        ):
            # ---- load constants ----
            w1_sb = singles.tile([128, 2, R2], F32)
            w2_sb = singles.tile([128, 4, 9, M2], F32)
            gmat_sb = singles.tile([128, 128], F32)
            g1_sb = singles.tile([128, 2], F32)
            b1_sb = singles.tile([128, 2], F32)
            g2_sb = singles.tile([128, 4], F32)
            b2_sb = singles.tile([128, 4], F32)
            nc.sync.dma_start(w1_sb, w1t_ap)
            nc.sync.dma_start(w2_sb, w2t_ap)
            nc.sync.dma_start(gmat_sb, gmat_ap)
            nc.sync.dma_start(g1_sb, g1_ap)
            nc.sync.dma_start(b1_sb, b1_ap)
            nc.sync.dma_start(g2_sb, g2_ap)
            nc.sync.dma_start(b2_sb, b2_ap)
            eps_t = singles.tile([128, 1], F32)
            nc.vector.memset(eps_t, float(EPS))
            if use_f32r:
                # memset can't target f32r tiles (ISA); zero-fill via DVE
                # copy-with-cast from a persistent fp32 zeros tile instead.
                zt = singles.tile([128, 512], F32)
                nc.vector.memset(zt, 0.0)

            def zfill(dst):
                if not use_f32r:
                    nc.vector.memset(dst, 0.0)
                    return
                dims = dst.shape[1:]
                n = 1
                for d in dims:
                    n *= d
                src = zt[:, 0:n]
                if len(dims) == 3:
                    src = src.rearrange(
                        "p (a b c) -> p a b c", a=dims[0], b=dims[1], c=dims[2]
                    )
                nc.vector.tensor_copy(out=dst, in_=src)
            if use_f32r:
                w1_mm = singles.tile([128, 2, R2], F32R)
                w2_mm = singles.tile([128, 4, 9, M2], F32R)
                nc.vector.tensor_copy(out=w1_mm, in_=w1_sb)
                nc.vector.tensor_copy(out=w2_mm, in_=w2_sb)
            else:
                w1_mm, w2_mm = w1_sb, w2_sb

            def allreduce_stats(pack_sb, ncols, name):
                cin = dramp.tile([128, ncols], F32, tag=f"cin{name}")
                cout = dramp.tile([128, ncols], F32, tag=f"cout{name}")
                nc.gpsimd.dma_start(cin, pack_sb)
                nc.gpsimd.collective_compute(
                    "AllReduce",
                    mybir.AluOpType.add,
                    replica_groups=groups,
                    ins=[cin.opt()],
                    outs=[cout.opt()],
                )
                rhs = singles.tile([128, ncols], F32, tag=f"rhs{name}")
                nc.sync.dma_start(rhs, cout)
                # group-average via 0/1(/32) matrix: also broadcasts back to rows
                ps = psum.tile([128, 512], F32, tag="ps")
                nc.tensor.matmul(
                    ps[:, 0:ncols], lhsT=gmat_sb, rhs=rhs, start=True, stop=True
                )
                statg = singles.tile([128, ncols // 2, 2], F32, tag=f"statg{name}")
                nc.scalar.copy(out=statg, in_=ps[:, 0:ncols])
                return statg

            # ================= Phase A: BN1 stats over x =================
            cha = 2048 if px % 2048 == 0 else 512
            nch = px // cha
            nsg = px // 512
            stats1 = singles.tile([128, 2, nsg, 6], F32)
            with nc.named_scope("phaseA"):
                for b in range(2):
                    for ci in range(nch):
                        xt = pA.tile([128, cha], F32, tag="xa_chunk")
                        nc.sync.dma_start(
                            xt, x_ap[b * 128 : (b + 1) * 128, ci * cha : (ci + 1) * cha]
                        )
                        for j in range(cha // 512):
                            nc.vector.bn_stats(
                                out=stats1[:, b, ci * (cha // 512) + j, :],
                                in_=xt[:, j * 512 : (j + 1) * 512],
                            )
                mv1 = singles.tile([128, 2, 2], F32)
                for b in range(2):
                    nc.vector.bn_aggr(out=mv1[:, b, :], in_=stats1[:, b, :, :])
                # pack (mean, E2) per row
                pk1 = singles.tile([128, 2, 2], F32)
                for b in range(2):
                    nc.vector.tensor_copy(out=pk1[:, b, 0:1], in_=mv1[:, b, 0:1])
                    nc.vector.tensor_tensor(
                        out=pk1[:, b, 1:2], in0=mv1[:, b, 0:1], in1=mv1[:, b, 0:1],
                        op=mybir.AluOpType.mult,
                    )
                    nc.vector.tensor_tensor(
                        out=pk1[:, b, 1:2], in0=pk1[:, b, 1:2], in1=mv1[:, b, 1:2],
                        op=mybir.AluOpType.add,
                    )
            with nc.named_scope("sync1"):
                statg1 = allreduce_stats(pk1, 4, "1")
                scale1, shift1 = _affine_from_stats(nc, singles, statg1, g1_sb, b1_sb, 2, eps_t)

            # ================= Phase B: conv1 (1x1) + BN2 stats =================
            out1_d = dramp.tile([4, 128, px], mmdt)
            chb = 1024 if px % 1024 == 0 else 512
            nb = px // chb
            sub = chb // 512
            stats2 = singles.tile([128, 4, nsg, 6], F32)
            ctxB = nc.named_scope("phaseB"); ctxB.__enter__()
            for obi in range(nb):
                c0 = obi * chb
                xa = pB.tile([128, 2, chb], F32, tag="xa")
                for b in range(2):
                    nc.sync.dma_start(
                        xa[:, b, :], x_ap[b * 128 : (b + 1) * 128, c0 : c0 + chb]
                    )
                ya = pB.tile([128, 2, chb], mmdt, tag="ya")
                for b in range(2):
                    if use_silu:
                        nc.scalar.activation(
                            out=ya[:, b, :], in_=xa[:, b, :], func=AF.Silu,
                            bias=shift1[:, b : b + 1], scale=scale1[:, b : b + 1],
                        )
                    else:
                        ta = pB.tile([128, chb], F32, tag="ta")
                        nc.vector.tensor_scalar(
                            out=ya[:, b, :], in0=xa[:, b, :],
                            scalar1=scale1[:, b : b + 1], scalar2=shift1[:, b : b + 1],
                            op0=mybir.AluOpType.mult, op1=mybir.AluOpType.add,
                        )
                        nc.scalar.activation(out=ta, in_=ya[:, b, :], func=AF.Sigmoid)
                        nc.vector.tensor_tensor(
                            out=ya[:, b, :], in0=ya[:, b, :], in1=ta,
                            op=mybir.AluOpType.mult,
                        )
                for m in range(4):
                    o1t = pB1.tile([128, chb], mmdt, tag="o1t")
                    pss = [psum.tile([128, 512], F32, tag="ps", name=f"psb{j}") for j in range(sub)]
                    for k in range(2):
                        for j in range(sub):
                            nc.tensor.matmul(
                                pss[j],
                                lhsT=w1_mm[:, k, m * 128 : (m + 1) * 128],
                                rhs=ya[:, k, j * 512 : (j + 1) * 512],
                                start=(k == 0), stop=(k == 1),
                            )
                    for j in range(sub):
                        dst = o1t[:, j * 512 : (j + 1) * 512]
                        if m % 2 == 0:
                            nc.scalar.copy(out=dst, in_=pss[j])
                        else:
                            nc.vector.tensor_copy(out=dst, in_=pss[j])
                        nc.vector.bn_stats(
                            out=stats2[:, m, obi * sub + j, :], in_=pss[j]
                        )
                    nc.gpsimd.dma_start(out1_d[m][:, c0 : c0 + chb], o1t)
            mv2 = singles.tile([128, 4, 2], F32)
            pk2 = singles.tile([128, 4, 2], F32)
            for m in range(4):
                nc.vector.bn_aggr(out=mv2[:, m, :], in_=stats2[:, m, :, :])
                nc.vector.tensor_copy(out=pk2[:, m, 0:1], in_=mv2[:, m, 0:1])
                nc.vector.tensor_tensor(
                    out=pk2[:, m, 1:2], in0=mv2[:, m, 0:1], in1=mv2[:, m, 0:1],
                    op=mybir.AluOpType.mult,
                )
                nc.vector.tensor_tensor(
                    out=pk2[:, m, 1:2], in0=pk2[:, m, 1:2], in1=mv2[:, m, 1:2],
                    op=mybir.AluOpType.add,
                )
            ctxB.__exit__(None, None, None)
            with nc.named_scope("sync2"):
                statg2 = allreduce_stats(pk2, 8, "2")
                scale2, shift2 = _affine_from_stats(nc, singles, statg2, g2_sb, b2_sb, 4, eps_t)

            # ================= Phase C: conv2 (3x3) =================
            G = 8
            ng = h // G
            wp = w + 2
            ctxC = nc.named_scope("phaseC"); ctxC.__enter__()
            for g in range(ng):
                h0 = g * G
                lo = h0 - 1
                win = pC.tile([128, 4, G + 2, wp], mmdt, tag="win")
                zfill(win[:, :, :, 0:1])
                zfill(win[:, :, :, w + 1 : w + 2])
                rs = max(h0 - 1, 0)
                re = min(h0 + G + 1, h)
                nr = re - rs
                s0 = rs - lo
                if s0 > 0:
                    zfill(win[:, :, 0:1, 1 : w + 1])
                if re < h0 + G + 1:
                    zfill(win[:, :, G + 1 : G + 2, 1 : w + 1])
                for kb in range(4):
                    src = out1_d[kb].rearrange("p (hh ww) -> p hh ww", ww=w)
                    nc.sync.dma_start(
                        win[:, kb, s0 : s0 + nr, 1 : w + 1], src[:, rs:re, :]
                    )
                for kb in range(4):
                    reg = win[:, kb, s0 : s0 + nr, 1 : w + 1]
                    if use_silu:
                        nc.scalar.activation(
                            out=reg, in_=reg, func=AF.Silu,
                            bias=shift2[:, kb : kb + 1], scale=scale2[:, kb : kb + 1],
                        )
                    else:
                        tsc = pC.tile([128, G + 2, w], F32, tag="tsc")
                        nc.vector.tensor_scalar(
                            out=reg, in0=reg,
                            scalar1=scale2[:, kb : kb + 1], scalar2=shift2[:, kb : kb + 1],
                            op0=mybir.AluOpType.mult, op1=mybir.AluOpType.add,
                        )
                        nc.scalar.activation(
                            out=tsc[:, s0 : s0 + nr, :], in_=reg, func=AF.Sigmoid
                        )
                        nc.vector.tensor_tensor(
                            out=reg, in0=reg, in1=tsc[:, s0 : s0 + nr, :],
                            op=mybir.AluOpType.mult,
                        )
                pss = [psum.tile([128, 4, w], F32, tag="ps", name=f"psc{hh}") for hh in range(2)]
                for kb in range(4):
                    for tap in range(9):
                        dy, dx = tap // 3, tap % 3
                        for half in range(2):
                            r0 = half * 4 + dy
                            nc.tensor.matmul(
                                pss[half],
                                lhsT=w2_mm[:, kb, tap, :],
                                rhs=win[:, kb, r0 : r0 + 4, dx : dx + w],
                                start=(kb == 0 and tap == 0),
                                stop=(kb == 3 and tap == 8),
                            )
                for half in range(2):
                    obt = pC2.tile([128, 4 * w], F32, tag="obt")
                    if half == 0:
                        nc.scalar.copy(out=obt, in_=pss[half])
                    else:
                        nc.vector.tensor_copy(out=obt, in_=pss[half])
                    p0 = (h0 + half * 4) * w
                    nc.gpsimd.dma_start(out2_ap[:, p0 : p0 + 4 * w], obt)
            ctxC.__exit__(None, None, None)

    nc.compile()
    return nc


# ---------------- host side ----------------

_QCOMP = [[0, 1, 2, 3], [1, 0, 3, 2], [2, 3, 0, 1], [3, 2, 1, 0]]
_QSIGN = [[1, -1, -1, -1], [1, 1, -1, 1], [1, 1, 1, -1], [1, -1, 1, 1]]


def hamilton_big(wq):
    """(4, O, C, kh, kw) -> (O*4, C*4, kh, kw) real block matrix."""
    wq = np.asarray(wq, np.float32)
    _, O, C = wq.shape[:3]
    rest = wq.shape[3:]
    big = np.zeros((O, 4, C, 4) + rest, np.float32)
    for qo in range(4):
        for qi in range(4):
            big[:, qo, :, qi] = _QSIGN[qo][qi] * wq[_QCOMP[qo][qi]]
    return big.reshape((O * 4, C * 4) + rest)


def make_host_inputs(w1, w2, gamma1, beta1, gamma2, beta2, n_cores=N_CORES):
    w1 = np.asarray(w1, np.float32)
    w2 = np.asarray(w2, np.float32)
    big1 = hamilton_big(w1)[:, :, 0, 0]            # (512, 256)
    big2 = hamilton_big(w2)                        # (128, 512, 3, 3)
    # w1t[p, kb, m] = big1[m, kb*128+p]
    w1t = np.ascontiguousarray(big1.T.reshape(2, 128, R2).transpose(1, 0, 2))
    # w2t[p, kb, tap, m] = big2[m, kb*128+p, dy, dx]
    w2t = np.ascontiguousarray(
        big2.transpose(1, 2, 3, 0).reshape(4, 128, 9, M2).transpose(1, 0, 2, 3)
    )
    gmat = (np.kron(np.eye(32, dtype=np.float32), np.ones((4, 4), np.float32))
            / (4.0 * n_cores))
    g1 = np.ascontiguousarray(
        np.repeat(np.asarray(gamma1, np.float32), 4).reshape(2, 128).T)
    b1 = np.ascontiguousarray(
        np.repeat(np.asarray(beta1, np.float32), 4).reshape(2, 128).T)
    g2 = np.ascontiguousarray(
        np.repeat(np.asarray(gamma2, np.float32), 4).reshape(4, 128).T)
    b2 = np.ascontiguousarray(
        np.repeat(np.asarray(beta2, np.float32), 4).reshape(4, 128).T)
    return dict(w1t=w1t, w2t=w2t, gmat=gmat, g1=g1, b1=b1, g2=g2, b2=b2)


_NC_CACHE = {}


def _get_nc(key=("hw",), **kw):
    if key not in _NC_CACHE:
        _NC_CACHE[key] = build_nc(**kw)
    return _NC_CACHE[key]


def run(x, gamma1, beta1, w1, gamma2, beta2, w2, trace=False, use_f32r=False):
    """Returns (full_output, BassKernelResults)."""
    x = np.asarray(x, np.float32)
    B = x.shape[0]
    assert x.shape == (B, C1, Q, H, W) and B == N_CORES
    const = make_host_inputs(w1, w2, gamma1, beta1, gamma2, beta2, N_CORES)
    in_maps = [
        {"x": np.ascontiguousarray(x[b].reshape(R1, H * W)), **const}
        for b in range(B)
    ]
    nc = _get_nc(key=("hw", use_f32r), use_f32r=use_f32r)
    res = run_bass_kernel_spmd(nc, in_maps, list(range(N_CORES)), trace=trace)
    out = np.empty((B, C1 + O2, Q, H, W), np.float32)
    out[:, :C1] = x
    for b in range(B):
        out[b, C1:] = res.results[b]["out2"].reshape(O2, Q, H, W)
    return out, res


def kernel(x, gamma1, beta1, w1, gamma2, beta2, w2):
    out, _ = run(x, gamma1, beta1, w1, gamma2, beta2, w2, trace=False)
    return out


# revision 16
# speedup vs baseline: 1.0017x; 1.0017x over previous
"""Trainium2 Bass kernel for nn_BottleneckBlock (quaternion bottleneck block).

Strategy: data-parallel over batch (B=8 -> 8 NeuronCores, 1 image each).
Per core, three phases in ONE NEFF:
  A: stream x, per-(channel,component)-row mean/E[x^2] via bn_stats/bn_aggr,
     AllReduce tiny stats across cores, fold gamma/beta -> per-row affine.
  B: stream x again, fused BN1-affine+SiLU on ScalarE, 1x1 quaternion conv as
     matmuls (Hamilton block matrix precomputed on host), write out1 to DRAM
     while accumulating BN2 stats; AllReduce, fold -> affine2.
  C: sliding row-window over out1 with zero-padded columns, fused
     BN2-affine+SiLU, 3x3 quaternion conv as 9 shifted matmuls accumulating
     in PSUM, write out2.
Host assembles concat([x, out2]) (pure data movement).
"""

import numpy as np

import concourse.bacc as bacc
import concourse.tile as tile
from concourse import mybir
from concourse.bass_utils import run_bass_kernel_spmd

F32 = mybir.dt.float32
F32R = mybir.dt.float32r
AF = mybir.ActivationFunctionType
EPS = 1e-5

N_CORES = 8
C1 = 64          # input quaternion channels
Q = 4
INTER = 128      # intermediate quaternion channels (out_planes*4)
O2 = 32          # output quaternion channels
R1 = C1 * Q      # 256 rows of x
R2 = INTER * Q   # 512 rows of out1
M2 = O2 * Q      # 128 rows of out2
H = W = 128


def enable_ldw_opt():
    """Rewrite walrus's --enable-ldw-opt=false to true (dedupes repeated
    identical LDWEIGHTS; our matmul order repeats weights back-to-back)."""
    import concourse.bass_utils as _bu

    if getattr(_bu, "_ldw_patched", False):
        return
    _orig = _bu.run_command

    def _patched(argv, **kw):
        argv = [
            "--enable-ldw-opt=true" if a == "--enable-ldw-opt=false" else a
            for a in argv
        ]
        return _orig(argv, **kw)

    _bu.run_command = _patched
    _bu._ldw_patched = True


def _affine_from_stats(nc, pool, statg, g_sb, b_sb, nb, eps_t):
    """statg: [128, nb, 2] group-averaged (mean, E[x^2]) per row.
    Returns (scale, shift) [128, nb] tiles with scale=gamma*rsqrt(var+eps),
    shift=beta-mean*scale. rsqrt = ACT sqrt + DVE reciprocal + 2 Newton steps
    (ACT sqrt alone has a loose precision budget)."""
    mean = statg[:, :, 0]
    e2 = statg[:, :, 1]
    vpe = pool.tile([128, nb], F32, tag=f"vpe{nb}")
    tmp = pool.tile([128, nb], F32, tag=f"ntmp{nb}")
    r = pool.tile([128, nb], F32, tag=f"nr{nb}")
    scale = pool.tile([128, nb], F32, tag=f"scale{nb}")
    shift = pool.tile([128, nb], F32, tag=f"shift{nb}")
    # vpe = E2 - mean^2 + eps
    nc.vector.tensor_tensor(out=tmp, in0=mean, in1=mean, op=mybir.AluOpType.mult)
    nc.vector.tensor_tensor(out=vpe, in0=e2, in1=tmp, op=mybir.AluOpType.subtract)
    nc.scalar.activation(out=r, in_=vpe, func=AF.Sqrt, bias=eps_t)
    nc.vector.tensor_scalar_add(out=vpe, in0=vpe, scalar1=float(EPS))
    nc.vector.reciprocal(out=r, in_=r)
    for _ in range(2):
        # r <- r * (1.5 - 0.5 * vpe * r^2)
        nc.vector.tensor_tensor(out=tmp, in0=r, in1=r, op=mybir.AluOpType.mult)
        nc.vector.tensor_tensor(out=tmp, in0=tmp, in1=vpe, op=mybir.AluOpType.mult)
        nc.vector.tensor_scalar(
            out=tmp, in0=tmp, scalar1=-0.5, scalar2=1.5,
            op0=mybir.AluOpType.mult, op1=mybir.AluOpType.add,
        )
        nc.vector.tensor_tensor(out=r, in0=r, in1=tmp, op=mybir.AluOpType.mult)
    nc.vector.tensor_tensor(out=scale, in0=g_sb, in1=r, op=mybir.AluOpType.mult)
    nc.vector.tensor_tensor(out=shift, in0=mean, in1=scale, op=mybir.AluOpType.mult)
    nc.vector.tensor_tensor(out=shift, in0=b_sb, in1=shift, op=mybir.AluOpType.subtract)
    return scale, shift


def build_nc(n_cores=N_CORES, h=H, w=W, use_silu=True, use_f32r=False):
    px = h * w
    assert px % 512 == 0
    mmdt = F32R if use_f32r else F32
    nc = bacc.Bacc("TRN2", target_bir_lowering=False, debug=False, num_devices=n_cores)

    x_ap = nc.dram_tensor("x", [R1, px], F32, kind="ExternalInput").ap()
    w1t_ap = nc.dram_tensor("w1t", [128, 2, R2], F32, kind="ExternalInput").ap()
    w2t_ap = nc.dram_tensor("w2t", [128, 4, 9, M2], F32, kind="ExternalInput").ap()
    gmat_ap = nc.dram_tensor("gmat", [128, 128], F32, kind="ExternalInput").ap()
    g1_ap = nc.dram_tensor("g1", [128, 2], F32, kind="ExternalInput").ap()
    b1_ap = nc.dram_tensor("b1", [128, 2], F32, kind="ExternalInput").ap()
    g2_ap = nc.dram_tensor("g2", [128, 4], F32, kind="ExternalInput").ap()
    b2_ap = nc.dram_tensor("b2", [128, 4], F32, kind="ExternalInput").ap()
    out2_ap = nc.dram_tensor("out2", [M2, px], F32, kind="ExternalOutput").ap()

    groups = [list(range(n_cores))]

    with tile.TileContext(nc) as tc:
        with (
            tc.tile_pool(name="singles", bufs=1) as singles,
            tc.tile_pool(name="pA", bufs=3) as pA,
            tc.tile_pool(name="pB", bufs=3) as pB,
            tc.tile_pool(name="pB1", bufs=6) as pB1,
            tc.tile_pool(name="pC", bufs=2) as pC,
            tc.tile_pool(name="pC2", bufs=4) as pC2,
            tc.tile_pool(name="psum", bufs=4, space="PSUM") as psum,
            tc.tile_pool(name="dram", bufs=1, space="DRAM") as dramp,
        ):
            # ---- load constants ----
            w1_sb = singles.tile([128, 2, R2], F32)
            w2_sb = singles.tile([128, 4, 9, M2], F32)
            gmat_sb = singles.tile([128, 128], F32)
            g1_sb = singles.tile([128, 2], F32)
            b1_sb = singles.tile([128, 2], F32)
            g2_sb = singles.tile([128, 4], F32)
            b2_sb = singles.tile([128, 4], F32)
            nc.sync.dma_start(w1_sb, w1t_ap)
            nc.sync.dma_start(w2_sb, w2t_ap)
            nc.sync.dma_start(gmat_sb, gmat_ap)
            nc.sync.dma_start(g1_sb, g1_ap)
            nc.sync.dma_start(b1_sb, b1_ap)
            nc.sync.dma_start(g2_sb, g2_ap)
            nc.sync.dma_start(b2_sb, b2_ap)
            eps_t = singles.tile([128, 1], F32)
            nc.vector.memset(eps_t, float(EPS))
            if use_f32r:
                # memset can't target f32r tiles (ISA); zero-fill via DVE
                # copy-with-cast from a persistent fp32 zeros tile instead.
                zt = singles.tile([128, 512], F32)
                nc.vector.memset(zt, 0.0)

            def zfill(dst):
                if not use_f32r:
                    nc.vector.memset(dst, 0.0)
                    return
                dims = dst.shape[1:]
                n = 1
                for d in dims:
                    n *= d
                src = zt[:, 0:n]
                if len(dims) == 3:
                    src = src.rearrange(
                        "p (a b c) -> p a b c", a=dims[0], b=dims[1], c=dims[2]
                    )
                nc.vector.tensor_copy(out=dst, in_=src)
            if use_f32r:
                w1_mm = singles.tile([128, 2, R2], F32R)
                w2_mm = singles.tile([128, 4, 9, M2], F32R)
                nc.vector.tensor_copy(out=w1_mm, in_=w1_sb)
                nc.vector.tensor_copy(out=w2_mm, in_=w2_sb)
            else:
                w1_mm, w2_mm = w1_sb, w2_sb

            def allreduce_stats(pack_sb, ncols, name):
                cin = dramp.tile([128, ncols], F32, tag=f"cin{name}")
                cout = dramp.tile([128, ncols], F32, tag=f"cout{name}")
                nc.gpsimd.dma_start(cin, pack_sb)
                nc.gpsimd.collective_compute(
                    "AllReduce",
                    mybir.AluOpType.add,
                    replica_groups=groups,
                    ins=[cin.opt()],
                    outs=[cout.opt()],
                )
                rhs = singles.tile([128, ncols], F32, tag=f"rhs{name}")
                nc.sync.dma_start(rhs, cout)
                # group-average via 0/1(/32) matrix: also broadcasts back to rows
                ps = psum.tile([128, 512], F32, tag="psC")
                nc.tensor.matmul(
                    ps[:, 0:ncols], lhsT=gmat_sb, rhs=rhs, start=True, stop=True
                )
                statg = singles.tile([128, ncols // 2, 2], F32, tag=f"statg{name}")
                nc.scalar.copy(out=statg, in_=ps[:, 0:ncols])
                return statg

            # ================= Phase A: BN1 stats over x =================
            cha = 2048 if px % 2048 == 0 else 512
            nch = px // cha
            nsg = px // 512
            stats1 = singles.tile([128, 2, nsg, 6], F32)
            with nc.named_scope("phaseA"):
                for b in range(2):
                    for ci in range(nch):
                        xt = pA.tile([128, cha], F32, tag="xa_chunk")
                        nc.sync.dma_start(
                            xt, x_ap[b * 128 : (b + 1) * 128, ci * cha : (ci + 1) * cha]
                        )
                        for j in range(cha // 512):
                            nc.vector.bn_stats(
                                out=stats1[:, b, ci * (cha // 512) + j, :],
                                in_=xt[:, j * 512 : (j + 1) * 512],
                            )
                mv1 = singles.tile([128, 2, 2], F32)
                for b in range(2):
                    nc.vector.bn_aggr(out=mv1[:, b, :], in_=stats1[:, b, :, :])
                # pack (mean, E2) per row
                pk1 = singles.tile([128, 2, 2], F32)
                for b in range(2):
                    nc.vector.tensor_copy(out=pk1[:, b, 0:1], in_=mv1[:, b, 0:1])
                    nc.vector.tensor_tensor(
                        out=pk1[:, b, 1:2], in0=mv1[:, b, 0:1], in1=mv1[:, b, 0:1],
                        op=mybir.AluOpType.mult,
                    )
                    nc.vector.tensor_tensor(
                        out=pk1[:, b, 1:2], in0=pk1[:, b, 1:2], in1=mv1[:, b, 1:2],
                        op=mybir.AluOpType.add,
                    )
            with nc.named_scope("sync1"):
                statg1 = allreduce_stats(pk1, 4, "1")
                scale1, shift1 = _affine_from_stats(nc, singles, statg1, g1_sb, b1_sb, 2, eps_t)

            # ================= Phase B: conv1 (1x1) + BN2 stats =================
            out1_d = dramp.tile([4, 128, px], mmdt)
            chb = 1024 if px % 1024 == 0 else 512
            nb = px // chb
            sub = chb // 512
            stats2 = singles.tile([128, 4, nsg, 6], F32)
            ctxB = nc.named_scope("phaseB"); ctxB.__enter__()
            for obi in range(nb):
                c0 = obi * chb
                xa = pB.tile([128, 2, chb], F32, tag="xa")
                for b in range(2):
                    nc.sync.dma_start(
                        xa[:, b, :], x_ap[b * 128 : (b + 1) * 128, c0 : c0 + chb]
                    )
                ya = pB.tile([128, 2, chb], mmdt, tag="ya")
                for b in range(2):
                    if use_silu:
                        nc.scalar.activation(
                            out=ya[:, b, :], in_=xa[:, b, :], func=AF.Silu,
                            bias=shift1[:, b : b + 1], scale=scale1[:, b : b + 1],
                        )
                    else:
                        ta = pB.tile([128, chb], F32, tag="ta")
                        nc.vector.tensor_scalar(
                            out=ya[:, b, :], in0=xa[:, b, :],
                            scalar1=scale1[:, b : b + 1], scalar2=shift1[:, b : b + 1],
                            op0=mybir.AluOpType.mult, op1=mybir.AluOpType.add,
                        )
                        nc.scalar.activation(out=ta, in_=ya[:, b, :], func=AF.Sigmoid)
                        nc.vector.tensor_tensor(
                            out=ya[:, b, :], in0=ya[:, b, :], in1=ta,
                            op=mybir.AluOpType.mult,
                        )
                for m in range(4):
                    o1t = pB1.tile([128, chb], mmdt, tag="o1t")
                    psb = psum.tile([128, chb], F32, tag="psB", bufs=2)
                    for k in range(2):
                        for j in range(sub):
                            nc.tensor.matmul(
                                psb[:, j * 512 : (j + 1) * 512],
                                lhsT=w1_mm[:, k, m * 128 : (m + 1) * 128],
                                rhs=ya[:, k, j * 512 : (j + 1) * 512],
                                start=(k == 0), stop=(k == 1),
                            )
                    nc.scalar.copy(out=o1t, in_=psb)
                    for j in range(sub):
                        nc.vector.bn_stats(
                            out=stats2[:, m, obi * sub + j, :],
                            in_=o1t[:, j * 512 : (j + 1) * 512],
                        )
                    nc.gpsimd.dma_start(out1_d[m][:, c0 : c0 + chb], o1t)
            mv2 = singles.tile([128, 4, 2], F32)
            pk2 = singles.tile([128, 4, 2], F32)
            for m in range(4):
                nc.vector.bn_aggr(out=mv2[:, m, :], in_=stats2[:, m, :, :])
                nc.vector.tensor_copy(out=pk2[:, m, 0:1], in_=mv2[:, m, 0:1])
                nc.vector.tensor_tensor(
                    out=pk2[:, m, 1:2], in0=mv2[:, m, 0:1], in1=mv2[:, m, 0:1],
                    op=mybir.AluOpType.mult,
                )
                nc.vector.tensor_tensor(
                    out=pk2[:, m, 1:2], in0=pk2[:, m, 1:2], in1=mv2[:, m, 1:2],
                    op=mybir.AluOpType.add,
                )
            ctxB.__exit__(None, None, None)
            with nc.named_scope("sync2"):
                statg2 = allreduce_stats(pk2, 8, "2")
                scale2, shift2 = _affine_from_stats(nc, singles, statg2, g2_sb, b2_sb, 4, eps_t)

            # ================= Phase C: conv2 (3x3) =================
            G = 8
            ng = h // G
            wp = w + 2
            ctxC = nc.named_scope("phaseC"); ctxC.__enter__()
            for g in range(ng):
                h0 = g * G
                lo = h0 - 1
                win = pC.tile([128, 4, G + 2, wp], mmdt, tag="win")
                zfill(win[:, :, :, 0:1])
                zfill(win[:, :, :, w + 1 : w + 2])
                rs = max(h0 - 1, 0)
                re = min(h0 + G + 1, h)
                nr = re - rs
                s0 = rs - lo
                if s0 > 0:
                    zfill(win[:, :, 0:1, 1 : w + 1])
                if re < h0 + G + 1:
                    zfill(win[:, :, G + 1 : G + 2, 1 : w + 1])
                for kb in range(4):
                    src = out1_d[kb].rearrange("p (hh ww) -> p hh ww", ww=w)
                    nc.sync.dma_start(
                        win[:, kb, s0 : s0 + nr, 1 : w + 1], src[:, rs:re, :]
                    )
                for kb in range(4):
                    reg = win[:, kb, s0 : s0 + nr, 1 : w + 1]
                    if use_silu:
                        nc.scalar.activation(
                            out=reg, in_=reg, func=AF.Silu,
                            bias=shift2[:, kb : kb + 1], scale=scale2[:, kb : kb + 1],
                        )
                    else:
                        tsc = pC.tile([128, G + 2, w], F32, tag="tsc")
                        nc.vector.tensor_scalar(
                            out=reg, in0=reg,
                            scalar1=scale2[:, kb : kb + 1], scalar2=shift2[:, kb : kb + 1],
                            op0=mybir.AluOpType.mult, op1=mybir.AluOpType.add,
                        )
                        nc.scalar.activation(
                            out=tsc[:, s0 : s0 + nr, :], in_=reg, func=AF.Sigmoid
                        )
                        nc.vector.tensor_tensor(
                            out=reg, in0=reg, in1=tsc[:, s0 : s0 + nr, :],
                            op=mybir.AluOpType.mult,
                        )
                pss = [psum.tile([128, 4, w], F32, tag="psC", name=f"psc{hh}") for hh in range(2)]
                for kb in range(4):
                    for tap in range(9):
                        dy, dx = tap // 3, tap % 3
                        for half in range(2):
                            r0 = half * 4 + dy
                            nc.tensor.matmul(
                                pss[half],
                                lhsT=w2_mm[:, kb, tap, :],
                                rhs=win[:, kb, r0 : r0 + 4, dx : dx + w],
                                start=(kb == 0 and tap == 0),
                                stop=(kb == 3 and tap == 8),
                            )
                for half in range(2):
                    obt = pC2.tile([128, 4 * w], F32, tag="obt")
                    if half == 0:
                        nc.scalar.copy(out=obt, in_=pss[half])
                    else:
                        nc.vector.tensor_copy(out=obt, in_=pss[half])
                    p0 = (h0 + half * 4) * w
                    nc.gpsimd.dma_start(out2_ap[:, p0 : p0 + 4 * w], obt)
            ctxC.__exit__(None, None, None)

    nc.compile()
    return nc


# ---------------- host side ----------------

_QCOMP = [[0, 1, 2, 3], [1, 0, 3, 2], [2, 3, 0, 1], [3, 2, 1, 0]]
_QSIGN = [[1, -1, -1, -1], [1, 1, -1, 1], [1, 1, 1, -1], [1, -1, 1, 1]]


def hamilton_big(wq):
    """(4, O, C, kh, kw) -> (O*4, C*4, kh, kw) real block matrix."""
    wq = np.asarray(wq, np.float32)
    _, O, C = wq.shape[:3]
    rest = wq.shape[3:]
    big = np.zeros((O, 4, C, 4) + rest, np.float32)
    for qo in range(4):
        for qi in range(4):
            big[:, qo, :, qi] = _QSIGN[qo][qi] * wq[_QCOMP[qo][qi]]
    return big.reshape((O * 4, C * 4) + rest)


def make_host_inputs(w1, w2, gamma1, beta1, gamma2, beta2, n_cores=N_CORES):
    w1 = np.asarray(w1, np.float32)
    w2 = np.asarray(w2, np.float32)
    big1 = hamilton_big(w1)[:, :, 0, 0]            # (512, 256)
    big2 = hamilton_big(w2)                        # (128, 512, 3, 3)
    # w1t[p, kb, m] = big1[m, kb*128+p]
    w1t = np.ascontiguousarray(big1.T.reshape(2, 128, R2).transpose(1, 0, 2))
    # w2t[p, kb, tap, m] = big2[m, kb*128+p, dy, dx]
    w2t = np.ascontiguousarray(
        big2.transpose(1, 2, 3, 0).reshape(4, 128, 9, M2).transpose(1, 0, 2, 3)
    )
    gmat = (np.kron(np.eye(32, dtype=np.float32), np.ones((4, 4), np.float32))
            / (4.0 * n_cores))
    g1 = np.ascontiguousarray(
        np.repeat(np.asarray(gamma1, np.float32), 4).reshape(2, 128).T)
    b1 = np.ascontiguousarray(
        np.repeat(np.asarray(beta1, np.float32), 4).reshape(2, 128).T)
    g2 = np.ascontiguousarray(
        np.repeat(np.asarray(gamma2, np.float32), 4).reshape(4, 128).T)
    b2 = np.ascontiguousarray(
        np.repeat(np.asarray(beta2, np.float32), 4).reshape(4, 128).T)
    return dict(w1t=w1t, w2t=w2t, gmat=gmat, g1=g1, b1=b1, g2=g2, b2=b2)


_NC_CACHE = {}


def _get_nc(key=("hw",), **kw):
    if key not in _NC_CACHE:
        _NC_CACHE[key] = build_nc(**kw)
    return _NC_CACHE[key]


def run(x, gamma1, beta1, w1, gamma2, beta2, w2, trace=False, use_f32r=False):
    """Returns (full_output, BassKernelResults)."""
    x = np.asarray(x, np.float32)
    B = x.shape[0]
    assert x.shape == (B, C1, Q, H, W) and B == N_CORES
    const = make_host_inputs(w1, w2, gamma1, beta1, gamma2, beta2, N_CORES)
    in_maps = [
        {"x": np.ascontiguousarray(x[b].reshape(R1, H * W)), **const}
        for b in range(B)
    ]
    nc = _get_nc(key=("hw", use_f32r), use_f32r=use_f32r)
    res = run_bass_kernel_spmd(nc, in_maps, list(range(N_CORES)), trace=trace)
    out = np.empty((B, C1 + O2, Q, H, W), np.float32)
    out[:, :C1] = x
    for b in range(B):
        out[b, C1:] = res.results[b]["out2"].reshape(O2, Q, H, W)
    return out, res


def kernel(x, gamma1, beta1, w1, gamma2, beta2, w2):
    out, _ = run(x, gamma1, beta1, w1, gamma2, beta2, w2, trace=False)
    return out
